# revision 2
# baseline (speedup 1.0000x reference)
"""GraphSAGE (3x SAGEConv + BN + relu, mean-pool, MLP head) -> [512, 2].

Distribution (graph/data parallel per the sharding hint): nodes are
partitioned into 8 contiguous ranges across the 8 NeuronCores. Each core
aggregates messages for its destination range (dma_gather from a replicated
activation table + CCE scatter-add), runs the SAGE transforms/BN locally
(PE/DVE/ACT), BN statistics and layer activations are combined with tiny
AllReduce / AllGather collectives, pooling + the MLP head run on-device, and
the full [512, 2] output is read back from core 0.

Falls back to a NumPy implementation if the device path fails.
"""
import numpy as np

N_NODES = 300000
N_EDGES = 600000
N_GRAPHS = 512
EPS = 1e-5

_STATE = {}


def _bass_kernel(inputs):
    import kernel_bass as KB
    from concourse.bass_utils import run_bass_kernel_spmd

    x = np.asarray(inputs["x"], np.float32)
    ei = np.asarray(inputs["edge_index"]).astype(np.int64)
    batch = np.asarray(inputs["batch"]).astype(np.int64)
    src, dst = ei[0], ei[1]

    cfg = _STATE.get("cfg")
    if cfg is None:
        cfg = KB.Cfg(N_NODES, N_EDGES, N_GRAPHS)
        _STATE["cfg"] = cfg

    in_maps, blocks, shared = KB.host_prepare(cfg, x, src, dst, batch)
    weights = KB.make_weight_inputs(cfg, inputs)

    key = tuple(blocks.tolist())
    if _STATE.get("key") != key:
        _STATE["nc"] = KB.build(cfg, blocks, weights)
        _STATE["key"] = key
    nc = _STATE["nc"]

    full_maps = []
    for m in range(KB.NCORES):
        im = {k: v for k, v in in_maps[m].items() if not k.startswith("_")}
        im.update(weights)
        im["remat"] = shared["remat"]
        im["iota72"] = shared["iota72"]
        im["ident"] = shared["ident"]
        full_maps.append(im)

    res = run_bass_kernel_spmd(nc, full_maps, core_ids=list(range(KB.NCORES)))
    out = np.asarray(res.results[0]["out"], np.float32)  # [2, G]
    return np.ascontiguousarray(out.T), res


def _numpy_kernel(inputs):
    x = np.asarray(inputs["x"], np.float32)
    ei = np.asarray(inputs["edge_index"]).astype(np.int64)
    batch = np.asarray(inputs["batch"]).astype(np.int64)
    src, dst = ei[0], ei[1]
    W = {k: np.asarray(v, np.float32) for k, v in inputs.items()
         if k not in ("x", "edge_index", "batch")}
    deg = np.bincount(dst, minlength=N_NODES).astype(np.float32)
    degc = np.maximum(deg, 1.0)[:, None]

    def sage(h, Wl, bl, Wr):
        agg = np.zeros((N_NODES, h.shape[1]), np.float32)
        np.add.at(agg, dst, h[src])
        agg /= degc
        return agg @ Wl.T + bl + h @ Wr.T

    def bn(z, g, b):
        mu = z.mean(axis=0)
        var = z.var(axis=0)
        return g * (z - mu) / np.sqrt(var + EPS) + b

    d_h, d_h2 = 128, 64
    b1 = W.get("b1", np.zeros(d_h, np.float32))
    b2 = W.get("b2", np.zeros(d_h, np.float32))
    b3 = W.get("b3", np.zeros(d_h2, np.float32))
    h = np.maximum(bn(sage(x, W["W1l"], b1, W["W1r"]), W["g1"], W["be1"]), 0.0)
    h = np.maximum(bn(sage(h, W["W2l"], b2, W["W2r"]), W["g2"], W["be2"]), 0.0)
    h = np.maximum(bn(sage(h, W["W3l"], b3, W["W3r"]), W["g3"], W["be3"]), 0.0)
    s = np.zeros((N_GRAPHS, h.shape[1]), np.float32)
    np.add.at(s, batch, h)
    cnt = np.bincount(batch, minlength=N_GRAPHS).astype(np.float32)
    pooled = s / np.maximum(cnt, 1.0)[:, None]
    z = np.maximum(pooled @ W["fc1_w"].T + W["fc1_b"], 0.0)
    return (z @ W["fc2_w"].T + W["fc2_b"]).astype(np.float32)


def kernel(**inputs):
    try:
        out, _ = _bass_kernel(inputs)
        return out
    except Exception as e:  # pragma: no cover - fallback safety
        import traceback
        traceback.print_exc()
        print(f"[kernel] bass path failed ({type(e).__name__}: {e}); numpy fallback")
        return _numpy_kernel(inputs)
